# revision 5
# baseline (speedup 1.0000x reference)
"""Per-frame RMS energy (STFT framing: n_fft=1024, hop=256, center/reflect pad)
over a [16, 1048576] f32 signal -> [16, 4096, 1] f32.

Trainium2 Bass/Tile kernel, data-parallel over batch across 8 NeuronCores
(2 signals per core). Each 1024-sample frame is exactly 4 consecutive
256-sample hop blocks, so we compute per-block sums of squares (one read of
every input byte -> memory-bound optimal), then a sliding sum of 4 plus
sqrt(mean). The reflect padding only affects 3 edge block sums, computed
separately from small slices.
"""

import numpy as np

import concourse.bacc as bacc
import concourse.mybir as mybir
import concourse.tile as tile
from concourse.bass_utils import run_bass_kernel_spmd

# Problem constants (self-contained; must match the grader's input spec)
B = 16                 # signals in the batch
T = 1048576            # samples per signal
N_FFT = 1024
HOP = 256
N_CORES = 8
SIG_PER_CORE = B // N_CORES   # 2
P = 128                       # SBUF partitions
NBLK = T // HOP               # 4096 hop blocks per signal
CPB = NBLK // P               # 32 block sums per partition
SPP = T // P                  # 8192 samples per partition row
NFRAMES = NBLK                # 4096 output frames per signal
CHUNK = 2048                  # samples per partition per DMA chunk (1 MiB tiles)
NCH = SPP // CHUNK            # 4 chunks per signal
GRP = CHUNK // HOP            # 8 block sums per chunk per partition

F32 = mybir.dt.float32
AF = mybir.ActivationFunctionType
AX = mybir.AxisListType
ADD = mybir.AluOpType.add


def build_bass():
    # Bacc (not plain Bass): its compile pipeline splits multi-sem waits into
    # event-semaphore instructions, which this walrus build requires.
    nc = bacc.Bacc()
    x = nc.dram_tensor("signal", [SIG_PER_CORE, T], F32, kind="ExternalInput")
    y = nc.dram_tensor("out", [SIG_PER_CORE, NFRAMES], F32, kind="ExternalOutput")

    xr = x[:, :].rearrange("b (p f) -> b p f", p=P)   # [2, 128, 8192]
    yr = y[:, :].rearrange("b (p c) -> b p c", p=P)   # [2, 128, 32]

    with tile.TileContext(nc) as tc:
        with (
            tc.tile_pool(name="inp", bufs=3) as inp_pool,
            tc.tile_pool(name="sq", bufs=3) as sq_pool,
            tc.tile_pool(name="ext", bufs=2) as ext_pool,
            tc.tile_pool(name="spec", bufs=2) as spec_pool,
            tc.tile_pool(name="small", bufs=2) as small_pool,
        ):
            for sig in range(SIG_PER_CORE):
                # ext[p, u] = s_pad[p*32 + u] for u in 0..34, where s_pad[b] is
                # the sum of squares of padded-signal block b. Interior blocks
                # (u in 2..33) are plain signal blocks; u 0,1,34 come from the
                # neighbor partition or the reflect-pad edge sums.
                ext = ext_pool.tile([P, 36], F32)

                # Main pass: load 1 MiB chunks, square on ACT, 256-block sums
                # on DVE straight into ext[:, 2:34].
                for c in range(NCH):
                    tin = inp_pool.tile([P, CHUNK], F32)
                    nc.sync.dma_start(
                        out=tin[:, :], in_=xr[sig, :, c * CHUNK : (c + 1) * CHUNK]
                    )
                    tsq = sq_pool.tile([P, CHUNK], F32)
                    nc.scalar.activation(out=tsq[:, :], in_=tin[:, :], func=AF.Square)
                    nc.vector.tensor_reduce(
                        out=ext[:, 2 + GRP * c : 2 + GRP * (c + 1)],
                        in_=tsq[:, :].rearrange("p (g k) -> p g k", k=HOP),
                        axis=AX.X,
                        op=ADD,
                    )

                # Reflect-pad edge sums (compute APs must start on a partition
                # quadrant, so everything runs on partition 0 and the right
                # edge scalar is DMA'd to partition 127 afterwards):
                #   s_pad[0]    = sum x[257:513]^2   -> ext[0, 0]
                #   s_pad[1]    = sum x[1:257]^2     -> ext[0, 1]
                #   s_pad[4098] = sum x[T-257:T-1]^2 -> ext[127, 34]
                spc = spec_pool.tile([P, 768], F32)
                spq = spec_pool.tile([P, 768], F32)
                spr = spec_pool.tile([P, 1], F32)
                nc.gpsimd.dma_start(out=spc[0:1, 0:512], in_=x[sig : sig + 1, 1:513])
                nc.gpsimd.dma_start(
                    out=spc[0:1, 512:768], in_=x[sig : sig + 1, T - 257 : T - 1]
                )
                nc.scalar.activation(out=spq[0:1, :], in_=spc[0:1, :], func=AF.Square)
                nc.vector.tensor_reduce(
                    out=ext[0:1, 1:2], in_=spq[0:1, 0:256], axis=AX.X, op=ADD
                )
                nc.vector.tensor_reduce(
                    out=ext[0:1, 0:1], in_=spq[0:1, 256:512], axis=AX.X, op=ADD
                )
                nc.vector.tensor_reduce(
                    out=spr[0:1, 0:1], in_=spq[0:1, 512:768], axis=AX.X, op=ADD
                )
                nc.gpsimd.dma_start(out=ext[127:128, 34:35], in_=spr[0:1, 0:1])

                # Cross-partition window boundaries (3 block sums per seam),
                # moved as two tiny SBUF->SBUF DMAs.
                nc.gpsimd.dma_start(out=ext[1:128, 0:2], in_=ext[0:127, 32:34])
                nc.gpsimd.dma_start(out=ext[0:127, 34:35], in_=ext[1:128, 2:3])

                # E[p, c] = ext[p, c] + ext[p, c+1] + ext[p, c+2] + ext[p, c+3]
                e1 = small_pool.tile([P, CPB], F32)
                e2 = small_pool.tile([P, CPB], F32)
                nc.vector.tensor_add(out=e1[:, :], in0=ext[:, 0:32], in1=ext[:, 1:33])
                nc.vector.tensor_add(out=e2[:, :], in0=ext[:, 2:34], in1=ext[:, 3:35])
                nc.vector.tensor_add(out=e1[:, :], in0=e1[:, :], in1=e2[:, :])
                ot = small_pool.tile([P, CPB], F32)
                nc.scalar.activation(
                    out=ot[:, :], in_=e1[:, :], func=AF.Sqrt, scale=1.0 / N_FFT
                )
                nc.gpsimd.dma_start(out=yr[sig, :, :], in_=ot[:, :])
    nc.finalize()
    return nc


_NC = None


def run(signal: np.ndarray, trace: bool = False):
    global _NC
    sig = np.ascontiguousarray(np.asarray(signal, dtype=np.float32))
    assert sig.shape == (B, T), sig.shape
    if _NC is None:
        _NC = build_bass()
    in_maps = [
        {"signal": np.ascontiguousarray(sig[k * SIG_PER_CORE : (k + 1) * SIG_PER_CORE])}
        for k in range(N_CORES)
    ]
    res = run_bass_kernel_spmd(_NC, in_maps, core_ids=list(range(N_CORES)), trace=trace)
    out = np.concatenate([r["out"] for r in res.results], axis=0)
    return out.reshape(B, NFRAMES, 1).astype(np.float32), res


def kernel(signal: np.ndarray) -> np.ndarray:
    out, _ = run(signal, trace=False)
    return out
